# revision 1
# baseline (speedup 1.0000x reference)
# kernel.py — Trainium2 Bass kernel for nn_Net_17188459119113 (quantized CNN).
#
# Pipeline (per reference.py):
#   xq = quant4(x); wq = quant4(conv_w)
#   y  = conv2d(xq, wq, VALID) + b; relu; maxpool 4x4/4; flatten
#   fq = quant4(flat); out = fq @ quant4(fc_w).T + fc_b
#
# Strategy: pure data-parallel over 8 NeuronCores (batch 8192 -> 1024/core).
# On device, everything runs in the integer domain (quantized values are
# small exact integers in fp16/fp32), with affine scales applied late:
#   - x is quantized on device via the fp32 round-to-nearest-even magic
#     constant trick (v + 1.5*2^23 - 1.5*2^23), scale folded as multiply.
#   - conv = banded matmul: K = (dj in 0..2) x (h in 0..27) = 84 partitions,
#     stationary = banded integer weights [84, 128] per M-chunk
#     (M = 16 oc x 8 i-rows), moving = 3 shifted copies of the image rows,
#     N = (16 batch x 24 j) = 384 columns per matmul.
#   - W-direction maxpool fused into the PSUM drain (vector reduce_max over
#     j-windows, free dim), output as fp16 integers (exact, |y| <= 441).
#   - H-direction pool after a 16-bit DMA xbar transpose, again reduce_max.
#   - bias+relu deferred past the (monotone) max pools; relu folded into
#     the FC quantization clamp.
#   - global flat-max via gpsimd partition reduce + AllReduce(max) across
#     the 8 cores; FC = 5 accumulating [128,10]x[128,128] fp16 matmuls.
# Output returned as [10, 1024] per core, transposed/concatenated on host.

import numpy as np

P = 128
B_CORE = 1024  # images per core
NB = 8  # b-blocks of 128 images
NCORES = 8
MAGIC = float(np.float32(12582912.0))  # 1.5 * 2**23: fp32 RNE rounding trick

_NC = None  # cached compiled Bass module (input-independent)


def _f32(v):
    return np.float32(v)


def _host_quant_scale(t):
    # mirrors reference _quant scale computation in fp32 arithmetic
    n = _f32(7.0)
    m = np.max(np.abs(t.astype(np.float32))).astype(np.float32)
    return _f32(_f32(m / n) + _f32(1e-8))


def _build_nc():
    import concourse.bass as bass
    import concourse.mybir as mybir
    from concourse import bacc, bass_isa
    from concourse.tile import TileContext

    f32 = mybir.dt.float32
    f16 = mybir.dt.float16
    AF = mybir.ActivationFunctionType
    OP = mybir.AluOpType

    # Bacc (not bare Bass): its compile() runs generate_event_semaphores /
    # move_matmul_waits_to_ldweights, which legalize multi-semaphore waits
    # down to the 1-wait-per-instruction hardware limit.
    nc = bacc.Bacc(None, num_devices=NCORES)

    x_in = nc.declare_dram_parameter("x", [P, 6272], f32, isOutput=False)
    w3_in = nc.declare_dram_parameter("w3", [84, 384], f16, isOutput=False)
    fw_in = nc.declare_dram_parameter("fw", [P, 50], f16, isOutput=False)
    cbf_in = nc.declare_dram_parameter("cbf", [P, 640], f32, isOutput=False)
    fb_in = nc.declare_dram_parameter("fb", [P, 1], f32, isOutput=False)
    scal_in = nc.declare_dram_parameter("scal", [P, 4], f32, isOutput=False)
    out_ext = nc.declare_dram_parameter("out", [10, B_CORE], f32, isOutput=True)

    xq_dram = nc.dram_tensor("xq_scratch", [B_CORE, 28, 28], f16)
    cc_in = nc.dram_tensor("cc_in", [1, 512], f32)
    cc_out = nc.dram_tensor("cc_out", [1, 512], f32, addr_space="Shared")

    with TileContext(nc, num_cores=NCORES) as tc:
        with tc.tile_pool(name="const", bufs=1) as cpool:
            w3sb = cpool.tile([84, 384], f16)
            fwsb = cpool.tile([P, 50], f16)
            cbf = cpool.tile([P, 640], f32)
            fb = cpool.tile([P, 1], f32)
            scal = cpool.tile([P, 4], f32)
            lmax = cpool.tile([P, 1], f32)
            magic = cpool.tile([P, 1], f32)
            # DVE-produced copies of small constants: consumers then need at
            # most one DVE semaphore + one DMA semaphore (walrus caps the
            # number of sync-wait commands per instruction).
            scal_a = cpool.tile([P, 4], f32)
            fb2 = cpool.tile([P, 1], f32)
            nc.vector.memset(magic[:, :], MAGIC)
            nc.sync.dma_start(out=w3sb[:, :], in_=w3_in[:, :])
            nc.sync.dma_start(out=fwsb[:, :], in_=fw_in[:, :])
            nc.sync.dma_start(out=cbf[:, :], in_=cbf_in[:, :])
            nc.sync.dma_start(out=fb[:, :], in_=fb_in[:, :])
            nc.sync.dma_start(out=scal[:, :], in_=scal_in[:, :])
            nc.vector.memset(lmax[:, :], -3.0e38)
            nc.vector.tensor_copy(out=scal_a[:, :], in_=scal[:, :])
            nc.vector.tensor_copy(out=fb2[:, :], in_=fb[:, :])

            flatr = []  # persistent per-b-block real (pre-relu) flat tiles
            with (
                tc.tile_pool(name="xq", bufs=1) as xqpool,
                tc.tile_pool(name="flatr", bufs=NB) as frpool,
                tc.tile_pool(name="small", bufs=2) as smpool,
            ):
              # ---------- Phase 1: quantize x to integers (fp16) ----------
              if True:
                xf = xqpool.tile([P, 6272], f32)
                t1 = xqpool.tile([P, 6272], f32)
                xq16 = xqpool.tile([P, 6272], f16)
                nc.sync.dma_start(out=xf[:, :], in_=x_in[:, :])
                nchunk = 4
                w = 6272 // nchunk
                for q in range(nchunk):
                    sl = slice(q * w, (q + 1) * w)
                    # t1 = x * (1/s_x) + MAGIC   (fma on ACT, RNE at int grid)
                    nc.scalar.activation(
                        out=t1[:, sl], in_=xf[:, sl], func=AF.Identity,
                        bias=magic[:, 0:1], scale=scal_a[:, 0:1],
                    )
                    # xq = t1 - MAGIC  -> round-to-nearest-even integers
                    nc.vector.tensor_scalar(
                        out=xq16[:, sl], in0=t1[:, sl],
                        scalar1=MAGIC, scalar2=None, op0=OP.subtract,
                    )
                xq_flat = xq_dram[:, :, :].rearrange("b h w -> (b h w)")
                nc.sync.dma_start(out=xq_flat, in_=xq16[:, :])
              with (
                tc.tile_pool(name="x3", bufs=2) as x3pool,
                tc.tile_pool(name="ps", bufs=2, space="PSUM") as pspool,
                tc.tile_pool(name="yph", bufs=2) as yphpool,
                tc.tile_pool(name="tr6", bufs=2) as trpool,
                tc.tile_pool(name="flati", bufs=2) as fipool,
              ):
                for bb in range(NB):
                    # ---------- Phase 2: conv (banded matmul) ----------
                    x3 = x3pool.tile([84, 3584], f16)
                    xv = xq_dram[bb * P:(bb + 1) * P, :, :]
                    for dj in range(3):
                        src = xv[:, :, dj:dj + 26].rearrange("b h w -> h b w")
                        dst = x3[28 * dj:28 * (dj + 1), :].rearrange(
                            "h (b w) -> h b w", w=28)[:, :, 0:26]
                        nc.sync.dma_start(out=dst, in_=src)

                    flati = fipool.tile([P, 640], f16)
                    nc.vector.memset(flati[:, 576:640], 0.0)
                    fr = frpool.tile([P, 640], f32)
                    flatr.append(fr)

                    x3v = x3[:, :].rearrange("p (b w) -> p b w", w=28)
                    # wait-ladder: one tiny PE matmul per x3 dj-block so the
                    # real matmuls never need >1 DMA semaphore wait (walrus
                    # caps sync-wait commands per compute instruction at 2).
                    ps0 = pspool.tile([P, 2048], f32, tag="ps")
                    for pbase in (0, 32, 64):  # in dj-blocks 0/1/2 resp.
                        nc.tensor.matmul(
                            out=ps0[0:1, 0:1],
                            lhsT=x3[pbase:pbase + 1, 0:1],
                            rhs=x3[pbase:pbase + 1, 0:1],
                            start=True, stop=True,
                        )
                    for c in range(3):
                        yph = yphpool.tile([P, 768], f16)
                        for bsq in range(2):
                            ps = pspool.tile([P, 2048], f32, tag="ps")
                            for g in range(4):
                                bs = bsq * 4 + g
                                rhs = x3v[:, bs * 16:(bs + 1) * 16, 0:24]
                                nc.tensor.matmul(
                                    out=ps[:, g * 512:g * 512 + 384],
                                    lhsT=w3sb[:, c * 128:(c + 1) * 128],
                                    rhs=rhs, start=True, stop=True,
                                )
                            # W-pool: reduce max over j-windows of 4
                            pin = ps[:, :].rearrange(
                                "p (g s) -> p g s", g=4)[:, :, 0:384].rearrange(
                                "p g (b jj u) -> p g b jj u", b=16, jj=6, u=4)
                            yout = yph[:, :].rearrange(
                                "p (jj bq g b) -> p bq g b jj",
                                jj=6, bq=2, g=4, b=16)[:, bsq]
                            nc.vector.tensor_reduce(
                                out=yout, in_=pin,
                                axis=mybir.AxisListType.X, op=OP.max,
                            )
                        # ---------- Phase 3: transpose + H-pool ----------
                        for jj in range(6):
                            tr = trpool.tile([P, 128], f16)
                            nc.scalar.dma_start(
                                out=tr[:, :],
                                in_=yph[:, jj * 128:(jj + 1) * 128],
                                transpose=True,
                            )
                            tin = tr[:, :].rearrange(
                                "p (oc t u) -> p oc t u", oc=16, t=2, u=4)
                            tout = flati[:, 0:576].rearrange(
                                "p (oc ii jj) -> p oc ii jj", oc=16, ii=6, jj=6
                            )[:, :, 2 * c:2 * c + 2, jj]
                            nc.vector.tensor_reduce(
                                out=tout, in_=tin,
                                axis=mybir.AxisListType.X, op=OP.max,
                            )

                    # real pre-relu flat values: fr = s_xw * flati + conv_bias
                    nc.vector.tensor_scalar(
                        out=fr[:, :], in0=flati[:, :],
                        scalar1=scal_a[:, 1:2], scalar2=None, op0=OP.mult,
                    )
                    nc.vector.tensor_tensor(fr[:, :], fr[:, :], cbf[:, :], OP.add)
                    # local running max (pre-relu; relu applied to the max later)
                    tmp = smpool.tile([P, 1], f32)
                    nc.vector.tensor_reduce(
                        out=tmp[:, :], in_=fr[:, 0:576],
                        axis=mybir.AxisListType.X, op=OP.max,
                    )
                    nc.vector.tensor_tensor(lmax[:, :], lmax[:, :], tmp[:, :], OP.max)

              # ---------- Phase 4: global scale via AllReduce(max) ----------
              lmr = smpool.tile([P, 1], f32, tag="lmr")
              nc.gpsimd.partition_all_reduce(
                  lmr[:, :], lmax[:, :], 128, bass_isa.ReduceOp.max)
              bc = smpool.tile([1, 512], f32, tag="bc")
              nc.vector.tensor_copy(
                  out=bc[:, :], in_=lmr[0:1, 0:1].to_broadcast((1, 512)))
              nc.gpsimd.dma_start(out=cc_in[:, :], in_=bc[:, :])
              nc.gpsimd.collective_compute(
                  "AllReduce", OP.max,
                  replica_groups=[list(range(NCORES))],
                  ins=[cc_in[:, :]], outs=[cc_out[:, :]],
              )
              gm = smpool.tile([1, 1], f32, tag="gm")
              nc.gpsimd.dma_start(out=gm[:, :], in_=cc_out[0:1, 0:1])
              gmb = smpool.tile([P, 1], f32, tag="gmb")
              nc.gpsimd.partition_broadcast(gmb[:, :], gm[:, :], channels=P)
              # s_f = relu(gmax)/7 + 1e-8 ; invsf = 1/s_f ; sprod = s_f*s_fw
              rg = smpool.tile([P, 1], f32, tag="rg")
              nc.scalar.activation(out=rg[:, :], in_=gmb[:, :], func=AF.Relu)
              sf = smpool.tile([P, 1], f32, tag="sf")
              nc.vector.tensor_scalar(
                  out=sf[:, :], in0=rg[:, :],
                  scalar1=float(np.float32(1.0) / np.float32(7.0)),
                  scalar2=float(np.float32(1e-8)),
                  op0=OP.mult, op1=OP.add,
              )
              invsf = smpool.tile([P, 1], f32, tag="invsf")
              nc.vector.reciprocal(out=invsf[:, :], in_=sf[:, :])
              zerot = smpool.tile([P, 1], f32, tag="zerot")
              nc.vector.memset(zerot[:, :], 0.0)
              sprod = smpool.tile([P, 1], f32, tag="sprod")
              nc.vector.tensor_scalar(
                  out=sprod[:, :], in0=sf[:, :],
                  scalar1=scal_a[:, 2:3], scalar2=None, op0=OP.mult,
              )

              # ---------- Phase 5: FC ----------
              with (
                  tc.tile_pool(name="fq", bufs=2) as fqpool,
                  tc.tile_pool(name="fqt", bufs=3) as fqtpool,
                  tc.tile_pool(name="psfc", bufs=2, space="PSUM") as pfcpool,
                  tc.tile_pool(name="outp", bufs=2) as outpool,
              ):
                  for bb in range(NB):
                      fr = flatr[bb]
                      qt = fqpool.tile([P, 640], f32, tag="qt")
                      # relu + scale to quant grid: qt = max(fr*invsf, 0)
                      # (invsf > 0, so this equals max(fr,0)*invsf)
                      nc.vector.tensor_scalar(
                          out=qt[:, :], in0=fr[:, :],
                          scalar1=invsf[:, 0:1], scalar2=zerot[:, 0:1],
                          op0=OP.mult, op1=OP.max,
                      )
                      # round to nearest-even integers via magic add/sub
                      nc.scalar.activation(
                          out=qt[:, :], in_=qt[:, :], func=AF.Identity,
                          bias=magic[:, 0:1], scale=1.0,
                      )
                      fq = fqpool.tile([P, 640], f16, tag="fq")
                      nc.vector.tensor_scalar(
                          out=fq[:, :], in0=qt[:, :],
                          scalar1=MAGIC, scalar2=None, op0=OP.subtract,
                      )
                      psfc = pfcpool.tile([10, 128], f32)
                      for ks in range(5):
                          fqt = fqtpool.tile([P, 128], f16)
                          nc.scalar.dma_start(
                              out=fqt[:, :],
                              in_=fq[:, ks * 128:(ks + 1) * 128],
                              transpose=True,
                          )
                          nc.tensor.matmul(
                              out=psfc[:, :],
                              lhsT=fwsb[:, ks * 10:(ks + 1) * 10],
                              rhs=fqt[:, :],
                              start=(ks == 0), stop=(ks == 4),
                          )
                      osb = outpool.tile([10, 128], f32)
                      nc.scalar.activation(
                          out=osb[:, :], in_=psfc[:, :], func=AF.Identity,
                          bias=fb2[0:10, 0:1], scale=sprod[0:10, 0:1],
                      )
                      nc.sync.dma_start(
                          out=out_ext[:, bb * 128:(bb + 1) * 128], in_=osb[:, :])

    # Bacc passes: legalize multi-sem waits, fuse nops, codegen ISA subclasses.
    nc.finalize()
    return nc


def _host_constants(x, conv_w, conv_b, fc_w, fc_b):
    s_x = _host_quant_scale(x)
    s_w = _host_quant_scale(conv_w)
    s_fw = _host_quant_scale(fc_w)
    kw = np.round(conv_w.astype(np.float32) / s_w).astype(np.float32)
    kfw = np.round(fc_w.astype(np.float32) / s_fw).astype(np.float32)

    # banded conv weight matrix: W3[(dj,h), c*128 + oc*8 + isub] = kw[oc, h-i, dj]
    w3 = np.zeros((84, 384), np.float32)
    for dj in range(3):
        for c in range(3):
            for isub in range(8):
                i = 8 * c + isub
                for di in range(3):
                    h = i + di
                    if h < 28:
                        for oc in range(16):
                            w3[28 * dj + h, c * 128 + oc * 8 + isub] = kw[oc, 0, di, dj]

    # FC weights: fw[p, ks*10 + cls] = kfw[cls, ks*128 + p] (zero-padded)
    fw = np.zeros((P, 50), np.float32)
    for ks in range(5):
        for p in range(P):
            k = ks * 128 + p
            if k < 576:
                fw[p, ks * 10:(ks + 1) * 10] = kfw[:, k]

    # broadcast conv-bias pattern over flat index k = oc*36 + ii*6 + jj
    cbf_row = np.zeros((640,), np.float32)
    for k in range(576):
        cbf_row[k] = conv_b[k // 36]
    cbf = np.tile(cbf_row[None, :], (P, 1)).astype(np.float32)

    fb = np.zeros((P, 1), np.float32)
    fb[:10, 0] = fc_b.astype(np.float32)

    inv_sx = _f32(_f32(1.0) / s_x)
    s_xw = _f32(s_x * s_w)
    scal = np.tile(
        np.array([inv_sx, s_xw, s_fw, 0.0], np.float32)[None, :], (P, 1))

    return {
        "w3": w3.astype(np.float16),
        "fw": fw.astype(np.float16),
        "cbf": cbf,
        "fb": fb,
        "scal": scal.astype(np.float32),
    }


def _get_nc():
    global _NC
    if _NC is None:
        _NC = _build_nc()
    return _NC


def kernel(x, conv_w, conv_b, fc_w, fc_b, _trace=False):
    from concourse.bass_utils import run_bass_kernel_spmd

    x = np.asarray(x, np.float32)
    consts = _host_constants(
        x, np.asarray(conv_w, np.float32), np.asarray(conv_b, np.float32),
        np.asarray(fc_w, np.float32), np.asarray(fc_b, np.float32))

    nc = _get_nc()
    in_maps = []
    for c in range(NCORES):
        shard = x[c * B_CORE:(c + 1) * B_CORE].reshape(P, 6272)
        m = {"x": np.ascontiguousarray(shard)}
        m.update(consts)
        in_maps.append(m)

    res = run_bass_kernel_spmd(nc, in_maps, list(range(NCORES)), trace=_trace)
    out = np.concatenate([r["out"].T for r in res.results], axis=0)
    if _trace:
        kernel._last_results = res
    return np.ascontiguousarray(out.astype(np.float32))



# revision 11
# speedup vs baseline: 1.5712x; 1.5712x over previous
# kernel.py — Trainium2 Bass kernel for nn_Net_17188459119113 (quantized CNN).
#
# Pipeline (per reference.py):
#   xq = quant4(x); wq = quant4(conv_w)
#   y  = conv2d(xq, wq, VALID) + b; relu; maxpool 4x4/4; flatten
#   fq = quant4(flat); out = fq @ quant4(fc_w).T + fc_b
#
# Data-parallel over 8 NeuronCores (1024 images/core). All heavy math in the
# integer domain (4-bit quantized values are small exact ints), scales applied
# as affine constants at the edges.
#
# v2 design (vs. the DMA-transpose baseline):
#  - Host supplies x in h-major layout [112=(bq4,h28), 7168=(b256,w28)] f32,
#    so the banded-conv moving tensor is built with 4 contiguous SBUF->SBUF
#    byte-shifted DMA copies per 128-image block (no DRAM round trip, no
#    strided 52B descriptor storms).
#  - Conv = fp8 DoubleRow matmuls (2 k-halves/partition, 0.5 cyc/col):
#    k=(dj,h)=84 packed as [42,2], m=128=(oc16,t2,u4) (i=8c+4t+u), n=384=
#    (b16,j24), 8 matmuls x 3 c-chunks per block.
#  - j-pool (max over jw=j%4) = DVE tensor_reduce straight out of PSUM.
#  - i-pool (max over u, a partition dim) via DVE StreamTranspose (32x32
#    blocks, SBUF->SBUF) + tensor_tensor max trees on the Pool engine
#    (free dim after the transpose). No DMA/PE transposes anywhere.
#  - Second StreamTranspose puts flat features back on partitions in a
#    jj-padded layout; quantization runs with per-partition conv-bias on ACT;
#    FC consumes the result directly as 3 accumulating fp8 DoubleRow matmuls
#    against host-permuted weight slabs (no FC transpose).
#  - Global flat-max via gpsimd partition reduce + AllReduce(max), as before.
# Output returned as [10, 1024] per core, transposed/concatenated on host.

import numpy as np

P = 128
B_CORE = 1024  # images per core
NB = 8  # b-blocks of 128 images
NCORES = 8
MAGIC = float(np.float32(12582912.0))  # 1.5 * 2**23: fp32 RNE rounding trick

_NC = None  # cached compiled Bass module (input-independent)


def _f32(v):
    return np.float32(v)


def _host_quant_scale(t):
    # mirrors reference _quant scale computation in fp32 arithmetic
    n = _f32(7.0)
    m = np.max(np.abs(t.astype(np.float32))).astype(np.float32)
    return _f32(_f32(m / n) + _f32(1e-8))


# ---- layout helpers (shared by host-const builder and kernel) ----
# conv PSUM partition order: p = oc*8 + t*4 + u  (i = 8c + 4t + u)
# flatT free layout per block: (c3, jh2, imgq4, [jl4, ocl4, t2]=32) = 768
# fqT (post 2nd stream transpose): partition = ocg*32 + (jl*8 + ocl*2 + t),
#   free = (c3, jh2, imgq4, a32); image = imgq*32 + a; oc = ocg*4 + ocl;
#   jj = jh*4 + jl (jl>=2 & jh=1 are pad slots); ii = 2c + t.


def _build_nc():
    import concourse.bass as bass
    import concourse.mybir as mybir
    from concourse import bacc, bass_isa
    from concourse.tile import TileContext

    f32 = mybir.dt.float32
    f16 = mybir.dt.float16
    f8 = mybir.dt.float8e4
    AF = mybir.ActivationFunctionType
    OP = mybir.AluOpType
    DR = mybir.MatmulPerfMode.DoubleRow

    nc = bacc.Bacc(None, num_devices=NCORES)

    # x in h-major layout: [112=(bq4,h28), 7168=(b256,w28)] f32
    x_in = nc.declare_dram_parameter("x", [112, 7168], f32, isOutput=False)
    # DoubleRow banded conv weights: [42, 2*384] fp8 (logical [42,2,(c3,m128)])
    w3_in = nc.declare_dram_parameter("w3", [42, 768], f8, isOutput=False)
    # permuted FC slabs: [128, 60] fp8 (logical [128,(q3,two2,cls10)])
    fw_in = nc.declare_dram_parameter("fw", [P, 60], f8, isOutput=False)
    # per-partition conv bias in int units (conv_b[oc]/s_xw), fqT partition map
    cbp_in = nc.declare_dram_parameter("cbp", [P, 1], f32, isOutput=False)
    fb_in = nc.declare_dram_parameter("fb", [P, 1], f32, isOutput=False)
    scal_in = nc.declare_dram_parameter("scal", [P, 4], f32, isOutput=False)
    out_ext = nc.declare_dram_parameter("out", [10, B_CORE], f32, isOutput=True)

    cc_in = nc.dram_tensor("cc_in", [1, 512], f32)
    cc_out = nc.dram_tensor("cc_out", [1, 512], f32, addr_space="Shared")

    with TileContext(nc, num_cores=NCORES) as tc:
        with tc.tile_pool(name="const", bufs=1) as cpool:
            w3sb = cpool.tile([42, 768], f8)
            fwsb = cpool.tile([P, 60], f8)
            cbp = cpool.tile([P, 1], f32)
            fb = cpool.tile([P, 1], f32)
            scal = cpool.tile([P, 4], f32)
            lmax = cpool.tile([P, 1], f32)
            magic = cpool.tile([P, 1], f32)
            scal_a = cpool.tile([P, 4], f32)  # DVE-copied (sem-wait hygiene)
            nc.vector.memset(magic[:, :], MAGIC)
            nc.sync.dma_start(out=w3sb[:, :], in_=w3_in[:, :])
            nc.sync.dma_start(out=fwsb[:, :], in_=fw_in[:, :])
            nc.sync.dma_start(out=cbp[:, :], in_=cbp_in[:, :])
            nc.sync.dma_start(out=fb[:, :], in_=fb_in[:, :])
            nc.sync.dma_start(out=scal[:, :], in_=scal_in[:, :])
            nc.vector.memset(lmax[:, :], -3.0e38)
            nc.vector.tensor_copy(out=scal_a[:, :], in_=scal[:, :])

            # xq8 padded by 8 cols so the dj-shifted x3 copies can overrun.
            xq8 = cpool.tile([112, 7176], f8)
            nc.vector.memset(xq8[:, 7168:7176], 0.0)

            rel = []  # per-block relu'd flat activations (int units), f32
            with tc.tile_pool(name="rel", bufs=NB) as relpool:
              # ---------- Phase 1: quantize x to fp8 integers ----------
              with tc.tile_pool(name="xf", bufs=2) as xfpool:
                nchunk = 4
                w = 7168 // nchunk  # 1792 els (64 images)
                for q in range(nchunk):
                    sl = slice(q * w, (q + 1) * w)
                    xf = xfpool.tile([112, w], f32)
                    t1 = xfpool.tile([112, w], f32)
                    nc.sync.dma_start(out=xf[:, :], in_=x_in[:, sl])
                    # t1 = x*(1/s_x) + MAGIC (ACT fma; RNE to int grid)
                    nc.scalar.activation(
                        out=t1[:, :], in_=xf[:, :], func=AF.Identity,
                        bias=magic[0:112, 0:1], scale=scal_a[0:112, 0:1],
                    )
                    # xq8 = t1 - MAGIC (exact small ints, cast to fp8)
                    nc.gpsimd.tensor_scalar(
                        out=xq8[:, sl], in0=t1[:, :],
                        scalar1=MAGIC, scalar2=None, op0=OP.subtract,
                    )

              # ---------- Phase 2+3: conv, 2D max-pool, per-block flat ----
              with (
                  tc.tile_pool(name="x3", bufs=2) as x3pool,
                  tc.tile_pool(name="ps", bufs=2, space="PSUM") as pspool,
                  tc.tile_pool(name="yj", bufs=2) as yjpool,
                  tc.tile_pool(name="yt", bufs=2) as ytpool,
                  tc.tile_pool(name="ft", bufs=2) as ftpool,
                  tc.tile_pool(name="sm", bufs=2) as smpool,
              ):
                for bb in range(NB):
                    q28 = (bb // 2) * 28
                    base = (bb % 2) * 3584
                    # x3d [42, (two2, b128, w28)+pad] fp8: dj-shifted copies
                    x3 = x3pool.tile([42, 7176], f8)
                    src = xq8[q28:q28 + 28, :]
                    # dj=0 -> (i2=0, kp 0..27)
                    nc.sync.dma_start(
                        out=x3[0:28, 0:3584], in_=src[:, base:base + 3584])
                    # dj=1, h 0..13 -> (i2=0, kp 28..41)
                    nc.sync.dma_start(
                        out=x3[28:42, 0:3584],
                        in_=src[0:14, base + 1:base + 1 + 3584])
                    # dj=1, h 14..27 -> (i2=1, kp 0..13)
                    nc.sync.dma_start(
                        out=x3[0:14, 3588:3588 + 3584],
                        in_=src[14:28, base + 1:base + 1 + 3584])
                    # dj=2, h 0..27 -> (i2=1, kp 14..41)
                    nc.sync.dma_start(
                        out=x3[14:42, 3588:3588 + 3584],
                        in_=src[:, base + 2:base + 2 + 3584])

                    # two-halves at stride 3588 (3584 data + 4B pad each)
                    x3v = x3[:, 0:7176].rearrange(
                        "p (two f) -> p two f", two=2)[:, :, 0:3584].rearrange(
                        "p two (b w) -> p two b w", w=28)
                    ft = ftpool.tile([P, 768], f16)
                    # zero pad slots (c, jh=1, imgq, jl in {2,3}, oclt)
                    nc.vector.memset(
                        ft[:, :].rearrange(
                            "p (c jh imgq jl oclt) -> p c jh imgq jl oclt",
                            c=3, jh=2, imgq=4, jl=4)[:, :, 1, :, 2:4, :],
                        0.0)

                    for c in range(3):
                        yj = yjpool.tile([P, 768], f16)
                        for bsq in range(2):
                            ps = pspool.tile([P, 2048], f32, tag="ps")
                            for g in range(4):
                                bs = bsq * 4 + g
                                # rhs [42, 2, 16, 24]: imgs bs*16.., j 0..23
                                rhs = x3v[:, :, bs * 16:(bs + 1) * 16, 0:24]
                                nc.tensor.matmul(
                                    out=ps[:, g * 512:g * 512 + 384],
                                    lhsT=w3sb[:, :].rearrange(
                                        "p (c two m) -> p c two m", c=3, two=2
                                    )[:, c],
                                    rhs=rhs, start=True, stop=True,
                                    perf_mode=DR,
                                )
                            # j-pool: max over jw=j%4, PSUM -> SBUF fp16
                            pin = ps[:, :].rearrange(
                                "p (g s) -> p g s", g=4)[:, :, 0:384].rearrange(
                                "p g (b jj jw) -> p g b jj jw", b=16, jj=6, jw=4)
                            yout = yj[:, :].rearrange(
                                "p (jj half g b) -> p half g b jj",
                                jj=6, half=2, g=4, b=16)[:, bsq]
                            nc.vector.tensor_reduce(
                                out=yout, in_=pin,
                                axis=mybir.AxisListType.X, op=OP.max,
                            )
                        # i-pool part 1: StreamTranspose 32x32 blocks
                        # yj [p=(oc16,t2,u4), (jj6, img128)] ->
                        # yt [p=(ocg4,a32), (jj6, imgq4, (ocl4,t2,u4)=32)]
                        yt = ytpool.tile([P, 768], f16)
                        nc.vector.transpose(out=yt[:, :], in_=yj[:, :])
                        # i-pool part 2: max over u (now free) on Pool engine
                        tv = yt[:, :].rearrange(
                            "p (jj imgq oclt u) -> p jj imgq oclt u",
                            jj=6, imgq=4, oclt=8)
                        t1 = smpool.tile([P, 384], f16, tag="t1")
                        t1v = t1[:, :].rearrange(
                            "p (jj imgq oclt s) -> p jj imgq oclt s",
                            jj=6, imgq=4, oclt=8)
                        nc.vector.tensor_tensor(
                            t1v[:, :, :, :, :], tv[:, :, :, :, 0:2],
                            tv[:, :, :, :, 2:4], OP.max)
                        # final u-max into flatT slots (jh-split for affine APs)
                        ftv = ft[:, :].rearrange(
                            "p (c jh imgq jl oclt) -> p c jh imgq jl oclt",
                            c=3, jh=2, imgq=4, jl=4)
                        t1a = t1[:, :].rearrange(
                            "p (jj imgq oclt s) -> p jj imgq oclt s",
                            jj=6, imgq=4, oclt=8)
                        # jh=0: jj 0..3 -> jl 0..3
                        nc.vector.tensor_tensor(
                            ftv[:, c, 0, :, :, :].rearrange(
                                "p imgq jl oclt -> p jl imgq oclt"),
                            t1a[:, 0:4, :, :, 0],
                            t1a[:, 0:4, :, :, 1], OP.max)
                        # jh=1: jj 4..5 -> jl 0..1
                        nc.vector.tensor_tensor(
                            ftv[:, c, 1, :, 0:2, :].rearrange(
                                "p imgq jl oclt -> p jl imgq oclt"),
                            t1a[:, 4:6, :, :, 0],
                            t1a[:, 4:6, :, :, 1], OP.max)

                    # 2nd StreamTranspose: features onto partitions
                    # ft [p=(ocg4,a32), (c,jh,imgq, phi32)] ->
                    # fqT [p=(ocg4,phi32), (c,jh,imgq, a32)]
                    fqt = ftpool.tile([P, 768], f16, tag="fqt")
                    nc.vector.transpose(out=fqt[:, :], in_=ft[:, :])
                    # relu(v + bias) in int units, f32 (exact; persists)
                    rb = relpool.tile([P, 768], f32)
                    rel.append(rb)
                    nc.scalar.activation(
                        out=rb[:, :], in_=fqt[:, :], func=AF.Relu,
                        bias=cbp[:, 0:1], scale=1.0,
                    )
                    # running local max (relu'd, so >= 0)
                    tmp = smpool.tile([P, 1], f32, tag="lm")
                    nc.vector.tensor_reduce(
                        out=tmp[:, :], in_=rb[:, :],
                        axis=mybir.AxisListType.X, op=OP.max,
                    )
                    nc.vector.tensor_tensor(
                        lmax[:, :], lmax[:, :], tmp[:, :], OP.max)

              # ---------- Phase 4: global scale via AllReduce(max) ----------
              with tc.tile_pool(name="ar", bufs=1) as arpool:
                lmr = arpool.tile([P, 1], f32)
                nc.gpsimd.partition_all_reduce(
                    lmr[:, :], lmax[:, :], 128, bass_isa.ReduceOp.max)
                bc = arpool.tile([1, 512], f32)
                nc.vector.tensor_copy(
                    out=bc[:, :], in_=lmr[0:1, 0:1].to_broadcast((1, 512)))
                nc.gpsimd.dma_start(out=cc_in[:, :], in_=bc[:, :])
                nc.gpsimd.collective_compute(
                    "AllReduce", OP.max,
                    replica_groups=[list(range(NCORES))],
                    ins=[cc_in[:, :]], outs=[cc_out[:, :]],
                )
                gm = arpool.tile([1, 1], f32)
                nc.gpsimd.dma_start(out=gm[:, :], in_=cc_out[0:1, 0:1])
                gmb = arpool.tile([P, 1], f32)
                nc.gpsimd.partition_broadcast(gmb[:, :], gm[:, :], channels=P)
                # gmax >= 0 already (relu'd, int units); to real units first.
                gmr = arpool.tile([P, 1], f32)
                nc.vector.tensor_scalar(
                    out=gmr[:, :], in0=gmb[:, :],
                    scalar1=scal_a[:, 1:2], scalar2=None, op0=OP.mult,
                )
                # s_f = gmax_real/7 + 1e-8  (matches reference fp32 math)
                sf = arpool.tile([P, 1], f32)
                nc.vector.tensor_scalar(
                    out=sf[:, :], in0=gmr[:, :],
                    scalar1=float(np.float32(1.0) / np.float32(7.0)),
                    scalar2=float(np.float32(1e-8)),
                    op0=OP.mult, op1=OP.add,
                )
                invsf = arpool.tile([P, 1], f32)
                nc.vector.reciprocal(out=invsf[:, :], in_=sf[:, :])
                # alpha = s_xw / s_f  (rel is in int units)
                alpha = arpool.tile([P, 1], f32)
                nc.vector.tensor_scalar(
                    out=alpha[:, :], in0=invsf[:, :],
                    scalar1=scal_a[:, 1:2], scalar2=None, op0=OP.mult,
                )
                # sprod = s_f * s_fw (output scale)
                sprod = arpool.tile([P, 1], f32)
                nc.vector.tensor_scalar(
                    out=sprod[:, :], in0=sf[:, :],
                    scalar1=scal_a[:, 2:3], scalar2=None, op0=OP.mult,
                )

                # ---------- Phase 5: quantize + FC ----------
                with (
                    tc.tile_pool(name="fq", bufs=2) as fqpool,
                    tc.tile_pool(name="psfc", bufs=2, space="PSUM") as pfcpool,
                    tc.tile_pool(name="outp", bufs=2) as outpool,
                ):
                    for bb in range(NB):
                        rb = rel[bb]
                        # r2 = rel*alpha + MAGIC  (ACT fma -> RNE round)
                        r2 = fqpool.tile([P, 768], f32, tag="r2")
                        nc.scalar.activation(
                            out=r2[:, :], in_=rb[:, :], func=AF.Identity,
                            bias=magic[:, 0:1], scale=alpha[:, 0:1],
                        )
                        fq8 = fqpool.tile([P, 768], f8, tag="fq8")
                        nc.vector.tensor_scalar(
                            out=fq8[:, :], in0=r2[:, :],
                            scalar1=MAGIC, scalar2=None, op0=OP.subtract,
                        )
                        psfc = pfcpool.tile([10, 128], f32)
                        fqv = fq8[:, :].rearrange(
                            "p (ch n) -> p ch n", ch=6)
                        fwv = fwsb[:, :].rearrange(
                            "p (ch cls) -> p ch cls", ch=6)
                        for ch in range(6):
                            nc.tensor.matmul(
                                out=psfc[:, :],
                                lhsT=fwv[:, ch],
                                rhs=fqv[:, ch],
                                start=(ch == 0), stop=(ch == 5),
                            )
                        osb = outpool.tile([10, 128], f32)
                        nc.scalar.activation(
                            out=osb[:, :], in_=psfc[:, :], func=AF.Identity,
                            bias=fb[0:10, 0:1], scale=sprod[0:10, 0:1],
                        )
                        nc.sync.dma_start(
                            out=out_ext[:, bb * 128:(bb + 1) * 128],
                            in_=osb[:, :])

    nc.finalize()
    return nc


def _host_constants(x, conv_w, conv_b, fc_w, fc_b):
    s_x = _host_quant_scale(x)
    s_w = _host_quant_scale(conv_w)
    s_fw = _host_quant_scale(fc_w)
    kw = np.round(conv_w.astype(np.float32) / s_w).astype(np.float32)
    kfw = np.round(fc_w.astype(np.float32) / s_fw).astype(np.float32)

    # Banded DoubleRow conv weights.
    # logical row r = 28*dj + h (84); packed (i2 = r // 42, kp = r % 42).
    # m column (per c): oc*8 + t*4 + u; i = 8c + 4t + u; di = h - i in [0,3).
    w3 = np.zeros((42, 2, 3, 128), np.float32)
    for dj in range(3):
        for h in range(28):
            r = 28 * dj + h
            i2, kp = divmod(r, 42)
            for c in range(3):
                for t in range(2):
                    for u in range(4):
                        i = 8 * c + 4 * t + u
                        di = h - i
                        if 0 <= di <= 2:
                            for oc in range(16):
                                w3[kp, i2, c, oc * 8 + t * 4 + u] = \
                                    kw[oc, 0, di, dj]
    # lhsT layout [42, (c, two, m)]: DoubleRow needs the two k-halves
    # contiguous (two-stride == m) per the s3_lw_dual_fp8 ISA restriction.
    w3 = w3.transpose(0, 2, 1, 3).reshape(42, 768)

    # FC slabs permuted to the fqT layout.
    # fqT partition p = ocg*32 + jl*8 + ocl*2 + t ; chunk ch = c*2 + jh.
    # feature flat idx (torch flatten) = oc*36 + ii*6 + jj,
    #   oc = ocg*4 + ocl, ii = 2c + t, jj = jh*4 + jl (pad if jj >= 6).
    fw = np.zeros((128, 3, 2, 10), np.float32)
    for p in range(128):
        ocg, rem = divmod(p, 32)
        jl, rem2 = divmod(rem, 8)
        ocl, t = divmod(rem2, 2)
        oc = ocg * 4 + ocl
        for ch in range(6):
            c, jh = divmod(ch, 2)
            jj = jh * 4 + jl
            if jj < 6:
                k = oc * 36 + (2 * c + t) * 6 + jj
                fw[p, c, jh, :] = kfw[:, k]
    fw = fw.reshape(128, 60)

    # per-partition conv bias in int units (pad rows 0)
    cbp = np.zeros((128, 1), np.float32)
    s_xw = _f32(s_x * s_w)
    for p in range(128):
        ocg, rem = divmod(p, 32)
        jl, rem2 = divmod(rem, 8)
        ocl, t = divmod(rem2, 2)
        oc = ocg * 4 + ocl
        cbp[p, 0] = _f32(conv_b[oc] / s_xw)

    fb = np.zeros((P, 1), np.float32)
    fb[:10, 0] = fc_b.astype(np.float32)

    inv_sx = _f32(_f32(1.0) / s_x)
    scal = np.tile(
        np.array([inv_sx, s_xw, s_fw, 0.0], np.float32)[None, :], (P, 1))

    import ml_dtypes
    return {
        "w3": w3.astype(ml_dtypes.float8_e4m3),
        "fw": fw.astype(ml_dtypes.float8_e4m3),
        "cbp": cbp,
        "fb": fb,
        "scal": scal.astype(np.float32),
    }


def _get_nc():
    global _NC
    if _NC is None:
        _NC = _build_nc()
    return _NC


def kernel(x, conv_w, conv_b, fc_w, fc_b, _trace=False):
    from concourse.bass_utils import run_bass_kernel_spmd

    x = np.asarray(x, np.float32)
    consts = _host_constants(
        x, np.asarray(conv_w, np.float32), np.asarray(conv_b, np.float32),
        np.asarray(fc_w, np.float32), np.asarray(fc_b, np.float32))

    nc = _get_nc()
    in_maps = []
    for cix in range(NCORES):
        shard = x[cix * B_CORE:(cix + 1) * B_CORE]  # [1024,1,28,28]
        # h-major: [bq4, h28, b256, w28] -> [112, 7168]
        xh = shard.reshape(4, 256, 28, 28).transpose(0, 2, 1, 3)
        m = {"x": np.ascontiguousarray(xh.reshape(112, 7168))}
        m.update(consts)
        in_maps.append(m)

    res = run_bass_kernel_spmd(nc, in_maps, list(range(NCORES)), trace=_trace)
    out = np.concatenate([r["out"].T for r in res.results], axis=0)
    if _trace:
        kernel._last_results = res
    return np.ascontiguousarray(out.astype(np.float32))


# revision 12
# speedup vs baseline: 2.3026x; 1.4655x over previous
# kernel.py — Trainium2 Bass kernel for nn_Net_17188459119113 (quantized CNN).
#
# Pipeline (per reference.py):
#   xq = quant4(x); wq = quant4(conv_w)
#   y  = conv2d(xq, wq, VALID) + b; relu; maxpool 4x4/4; flatten
#   fq = quant4(flat); out = fq @ quant4(fc_w).T + fc_b
#
# Data-parallel over 8 NeuronCores (1024 images/core). All heavy math in the
# integer domain (4-bit quantized values are small exact ints), scales applied
# as affine constants at the edges.
#
# v2 design (vs. the DMA-transpose baseline):
#  - Host supplies x in h-major layout [112=(bq4,h28), 7168=(b256,w28)] f32,
#    so the banded-conv moving tensor is built with 4 contiguous SBUF->SBUF
#    byte-shifted DMA copies per 128-image block (no DRAM round trip, no
#    strided 52B descriptor storms).
#  - Conv = fp8 DoubleRow matmuls (2 k-halves/partition, 0.5 cyc/col):
#    k=(dj,h)=84 packed as [42,2], m=128=(oc16,t2,u4) (i=8c+4t+u), n=384=
#    (b16,j24), 8 matmuls x 3 c-chunks per block.
#  - j-pool (max over jw=j%4) = DVE tensor_reduce straight out of PSUM.
#  - i-pool (max over u, a partition dim) via DVE StreamTranspose (32x32
#    blocks, SBUF->SBUF) + tensor_tensor max trees on the Pool engine
#    (free dim after the transpose). No DMA/PE transposes anywhere.
#  - Second StreamTranspose puts flat features back on partitions in a
#    jj-padded layout; quantization runs with per-partition conv-bias on ACT;
#    FC consumes the result directly as 3 accumulating fp8 DoubleRow matmuls
#    against host-permuted weight slabs (no FC transpose).
#  - Global flat-max via gpsimd partition reduce + AllReduce(max), as before.
# Output returned as [10, 1024] per core, transposed/concatenated on host.

import numpy as np

P = 128
B_CORE = 1024  # images per core
NB = 8  # b-blocks of 128 images
NCORES = 8
MAGIC = float(np.float32(12582912.0))  # 1.5 * 2**23: fp32 RNE rounding trick

_NC = None  # cached compiled Bass module (input-independent)


def _f32(v):
    return np.float32(v)


def _host_quant_scale(t):
    # mirrors reference _quant scale computation in fp32 arithmetic
    n = _f32(7.0)
    m = np.max(np.abs(t.astype(np.float32))).astype(np.float32)
    return _f32(_f32(m / n) + _f32(1e-8))


# ---- layout helpers (shared by host-const builder and kernel) ----
# conv PSUM partition order: p = oc*8 + t*4 + u  (i = 8c + 4t + u)
# flatT free layout per block: (c3, jh2, imgq4, [jl4, ocl4, t2]=32) = 768
# fqT (post 2nd stream transpose): partition = ocg*32 + (jl*8 + ocl*2 + t),
#   free = (c3, jh2, imgq4, a32); image = imgq*32 + a; oc = ocg*4 + ocl;
#   jj = jh*4 + jl (jl>=2 & jh=1 are pad slots); ii = 2c + t.


def _build_nc():
    import concourse.bass as bass
    import concourse.mybir as mybir
    from concourse import bacc, bass_isa
    from concourse.tile import TileContext

    f32 = mybir.dt.float32
    f16 = mybir.dt.float16
    f8 = mybir.dt.float8e4
    AF = mybir.ActivationFunctionType
    OP = mybir.AluOpType
    DR = mybir.MatmulPerfMode.DoubleRow

    nc = bacc.Bacc(None, num_devices=NCORES)

    # x in h-major layout: [112=(bq4,h28), 7168=(b256,w28)] f32
    x_in = nc.declare_dram_parameter("x", [112, 7168], f32, isOutput=False)
    # DoubleRow banded conv weights: [42, 2*384] fp8 (logical [42,2,(c3,m128)])
    w3_in = nc.declare_dram_parameter("w3", [42, 768], f8, isOutput=False)
    # permuted FC slabs: [128, 60] fp8 (logical [128,(q3,two2,cls10)])
    fw_in = nc.declare_dram_parameter("fw", [P, 60], f8, isOutput=False)
    # per-partition conv bias in int units (conv_b[oc]/s_xw), fqT partition map
    cbp_in = nc.declare_dram_parameter("cbp", [P, 1], f32, isOutput=False)
    fb_in = nc.declare_dram_parameter("fb", [P, 1], f32, isOutput=False)
    scal_in = nc.declare_dram_parameter("scal", [P, 4], f32, isOutput=False)
    out_ext = nc.declare_dram_parameter("out", [10, B_CORE], f32, isOutput=True)

    cc_in = nc.dram_tensor("cc_in", [1, 512], f32)
    cc_out = nc.dram_tensor("cc_out", [1, 512], f32, addr_space="Shared")

    with TileContext(nc, num_cores=NCORES) as tc:
        with tc.tile_pool(name="const", bufs=1) as cpool:
            w3sb = cpool.tile([42, 768], f8)
            fwsb = cpool.tile([P, 60], f8)
            cbp = cpool.tile([P, 1], f32)
            fb = cpool.tile([P, 1], f32)
            scal = cpool.tile([P, 4], f32)
            lmax = cpool.tile([P, 1], f32)
            magic = cpool.tile([P, 1], f32)
            scal_a = cpool.tile([P, 4], f32)  # DVE-copied (sem-wait hygiene)
            nc.vector.memset(magic[:, :], MAGIC)
            nc.sync.dma_start(out=w3sb[:, :], in_=w3_in[:, :])
            nc.sync.dma_start(out=fwsb[:, :], in_=fw_in[:, :])
            nc.sync.dma_start(out=cbp[:, :], in_=cbp_in[:, :])
            nc.sync.dma_start(out=fb[:, :], in_=fb_in[:, :])
            nc.sync.dma_start(out=scal[:, :], in_=scal_in[:, :])
            nc.vector.memset(lmax[:, :], -3.0e38)
            nc.vector.tensor_copy(out=scal_a[:, :], in_=scal[:, :])

            # xq8 padded by 8 cols so the dj-shifted x3 copies can overrun.
            xq8 = cpool.tile([112, 7176], f8)
            nc.vector.memset(xq8[:, 7168:7176], 0.0)

            rel = []  # per-block relu'd flat activations (int units), f32
            with tc.tile_pool(name="rel", bufs=NB) as relpool:
              # ---------- Phase 1: quantize x to fp8 integers ----------
              with tc.tile_pool(name="xf", bufs=2) as xfpool:
                nchunk = 4
                w = 7168 // nchunk  # 1792 els (64 images)
                for q in range(nchunk):
                    sl = slice(q * w, (q + 1) * w)
                    xf = xfpool.tile([112, w], f32)
                    t1 = xfpool.tile([112, w], f32)
                    nc.sync.dma_start(out=xf[:, :], in_=x_in[:, sl])
                    # t1 = x*(1/s_x) + MAGIC (ACT fma; RNE to int grid)
                    nc.scalar.activation(
                        out=t1[:, :], in_=xf[:, :], func=AF.Identity,
                        bias=magic[0:112, 0:1], scale=scal_a[0:112, 0:1],
                    )
                    # xq8 = t1 - MAGIC (exact small ints, cast to fp8).
                    # NOTE: must be DVE — gpsimd runs this ~17x slower.
                    nc.vector.tensor_scalar(
                        out=xq8[:, sl], in0=t1[:, :],
                        scalar1=MAGIC, scalar2=None, op0=OP.subtract,
                    )

              # ---------- Phase 2+3: conv, 2D max-pool, per-block flat ----
              with (
                  tc.tile_pool(name="x3", bufs=2) as x3pool,
                  tc.tile_pool(name="ps", bufs=2, space="PSUM") as pspool,
                  tc.tile_pool(name="yj", bufs=2) as yjpool,
                  tc.tile_pool(name="yt", bufs=2) as ytpool,
                  tc.tile_pool(name="ft", bufs=2) as ftpool,
                  tc.tile_pool(name="sm", bufs=2) as smpool,
              ):
                for bb in range(NB):
                    q28 = (bb // 2) * 28
                    base = (bb % 2) * 3584
                    # x3d [42, (two2, b128, w28)+pad] fp8: dj-shifted copies
                    x3 = x3pool.tile([42, 7176], f8)
                    src = xq8[q28:q28 + 28, :]
                    # dj=0 -> (i2=0, kp 0..27)
                    nc.sync.dma_start(
                        out=x3[0:28, 0:3584], in_=src[:, base:base + 3584])
                    # dj=1, h 0..13 -> (i2=0, kp 28..41)
                    nc.sync.dma_start(
                        out=x3[28:42, 0:3584],
                        in_=src[0:14, base + 1:base + 1 + 3584])
                    # dj=1, h 14..27 -> (i2=1, kp 0..13)
                    nc.sync.dma_start(
                        out=x3[0:14, 3588:3588 + 3584],
                        in_=src[14:28, base + 1:base + 1 + 3584])
                    # dj=2, h 0..27 -> (i2=1, kp 14..41)
                    nc.sync.dma_start(
                        out=x3[14:42, 3588:3588 + 3584],
                        in_=src[:, base + 2:base + 2 + 3584])

                    # two-halves at stride 3588 (3584 data + 4B pad each)
                    x3v = x3[:, 0:7176].rearrange(
                        "p (two f) -> p two f", two=2)[:, :, 0:3584].rearrange(
                        "p two (b w) -> p two b w", w=28)
                    ft = ftpool.tile([P, 768], f16)
                    # zero pad slots (c, jh=1, imgq, jl in {2,3}, oclt)
                    nc.vector.memset(
                        ft[:, :].rearrange(
                            "p (c jh imgq jl oclt) -> p c jh imgq jl oclt",
                            c=3, jh=2, imgq=4, jl=4)[:, :, 1, :, 2:4, :],
                        0.0)

                    for c in range(3):
                        yj = yjpool.tile([P, 768], f16)
                        for bsq in range(2):
                            ps = pspool.tile([P, 2048], f32, tag="ps")
                            for g in range(4):
                                bs = bsq * 4 + g
                                # rhs [42, 2, 16, 24]: imgs bs*16.., j 0..23
                                rhs = x3v[:, :, bs * 16:(bs + 1) * 16, 0:24]
                                nc.tensor.matmul(
                                    out=ps[:, g * 512:g * 512 + 384],
                                    lhsT=w3sb[:, :].rearrange(
                                        "p (c two m) -> p c two m", c=3, two=2
                                    )[:, c],
                                    rhs=rhs, start=True, stop=True,
                                    perf_mode=DR,
                                )
                            # j-pool: max over jw=j%4, PSUM -> SBUF fp16
                            pin = ps[:, :].rearrange(
                                "p (g s) -> p g s", g=4)[:, :, 0:384].rearrange(
                                "p g (b jj jw) -> p g b jj jw", b=16, jj=6, jw=4)
                            yout = yj[:, :].rearrange(
                                "p (jj half g b) -> p half g b jj",
                                jj=6, half=2, g=4, b=16)[:, bsq]
                            nc.vector.tensor_reduce(
                                out=yout, in_=pin,
                                axis=mybir.AxisListType.X, op=OP.max,
                            )
                        # i-pool part 1: StreamTranspose 32x32 blocks
                        # yj [p=(oc16,t2,u4), (jj6, img128)] ->
                        # yt [p=(ocg4,a32), (jj6, imgq4, (ocl4,t2,u4)=32)]
                        yt = ytpool.tile([P, 768], f16)
                        nc.vector.transpose(out=yt[:, :], in_=yj[:, :])
                        # i-pool part 2: max over u (now free) on Pool engine
                        tv = yt[:, :].rearrange(
                            "p (jj imgq oclt u) -> p jj imgq oclt u",
                            jj=6, imgq=4, oclt=8)
                        t1 = smpool.tile([P, 384], f16, tag="t1")
                        t1v = t1[:, :].rearrange(
                            "p (jj imgq oclt s) -> p jj imgq oclt s",
                            jj=6, imgq=4, oclt=8)
                        nc.vector.tensor_tensor(
                            t1v[:, :, :, :, :], tv[:, :, :, :, 0:2],
                            tv[:, :, :, :, 2:4], OP.max)
                        # final u-max into flatT slots (jh-split for affine APs)
                        ftv = ft[:, :].rearrange(
                            "p (c jh imgq jl oclt) -> p c jh imgq jl oclt",
                            c=3, jh=2, imgq=4, jl=4)
                        t1a = t1[:, :].rearrange(
                            "p (jj imgq oclt s) -> p jj imgq oclt s",
                            jj=6, imgq=4, oclt=8)
                        # jh=0: jj 0..3 -> jl 0..3
                        nc.vector.tensor_tensor(
                            ftv[:, c, 0, :, :, :].rearrange(
                                "p imgq jl oclt -> p jl imgq oclt"),
                            t1a[:, 0:4, :, :, 0],
                            t1a[:, 0:4, :, :, 1], OP.max)
                        # jh=1: jj 4..5 -> jl 0..1
                        nc.vector.tensor_tensor(
                            ftv[:, c, 1, :, 0:2, :].rearrange(
                                "p imgq jl oclt -> p jl imgq oclt"),
                            t1a[:, 4:6, :, :, 0],
                            t1a[:, 4:6, :, :, 1], OP.max)

                    # 2nd StreamTranspose: features onto partitions
                    # ft [p=(ocg4,a32), (c,jh,imgq, phi32)] ->
                    # fqT [p=(ocg4,phi32), (c,jh,imgq, a32)]
                    fqt = ftpool.tile([P, 768], f16, tag="fqt")
                    nc.vector.transpose(out=fqt[:, :], in_=ft[:, :])
                    # relu(v + bias) in int units, f32 (exact; persists)
                    rb = relpool.tile([P, 768], f32)
                    rel.append(rb)
                    nc.scalar.activation(
                        out=rb[:, :], in_=fqt[:, :], func=AF.Relu,
                        bias=cbp[:, 0:1], scale=1.0,
                    )
                    # running local max (relu'd, so >= 0)
                    tmp = smpool.tile([P, 1], f32, tag="lm")
                    nc.vector.tensor_reduce(
                        out=tmp[:, :], in_=rb[:, :],
                        axis=mybir.AxisListType.X, op=OP.max,
                    )
                    nc.vector.tensor_tensor(
                        lmax[:, :], lmax[:, :], tmp[:, :], OP.max)

              # ---------- Phase 4: global scale via AllReduce(max) ----------
              with tc.tile_pool(name="ar", bufs=1) as arpool:
                lmr = arpool.tile([P, 1], f32)
                nc.gpsimd.partition_all_reduce(
                    lmr[:, :], lmax[:, :], 128, bass_isa.ReduceOp.max)
                bc = arpool.tile([1, 512], f32)
                nc.vector.tensor_copy(
                    out=bc[:, :], in_=lmr[0:1, 0:1].to_broadcast((1, 512)))
                nc.gpsimd.dma_start(out=cc_in[:, :], in_=bc[:, :])
                nc.gpsimd.collective_compute(
                    "AllReduce", OP.max,
                    replica_groups=[list(range(NCORES))],
                    ins=[cc_in[:, :]], outs=[cc_out[:, :]],
                )
                gm = arpool.tile([1, 1], f32)
                nc.gpsimd.dma_start(out=gm[:, :], in_=cc_out[0:1, 0:1])
                gmb = arpool.tile([P, 1], f32)
                nc.gpsimd.partition_broadcast(gmb[:, :], gm[:, :], channels=P)
                # gmax >= 0 already (relu'd, int units); to real units first.
                gmr = arpool.tile([P, 1], f32)
                nc.vector.tensor_scalar(
                    out=gmr[:, :], in0=gmb[:, :],
                    scalar1=scal_a[:, 1:2], scalar2=None, op0=OP.mult,
                )
                # s_f = gmax_real/7 + 1e-8  (matches reference fp32 math)
                sf = arpool.tile([P, 1], f32)
                nc.vector.tensor_scalar(
                    out=sf[:, :], in0=gmr[:, :],
                    scalar1=float(np.float32(1.0) / np.float32(7.0)),
                    scalar2=float(np.float32(1e-8)),
                    op0=OP.mult, op1=OP.add,
                )
                invsf = arpool.tile([P, 1], f32)
                nc.vector.reciprocal(out=invsf[:, :], in_=sf[:, :])
                # alpha = s_xw / s_f  (rel is in int units)
                alpha = arpool.tile([P, 1], f32)
                nc.vector.tensor_scalar(
                    out=alpha[:, :], in0=invsf[:, :],
                    scalar1=scal_a[:, 1:2], scalar2=None, op0=OP.mult,
                )
                # sprod = s_f * s_fw (output scale)
                sprod = arpool.tile([P, 1], f32)
                nc.vector.tensor_scalar(
                    out=sprod[:, :], in0=sf[:, :],
                    scalar1=scal_a[:, 2:3], scalar2=None, op0=OP.mult,
                )

                # ---------- Phase 5: quantize + FC ----------
                with (
                    tc.tile_pool(name="fq", bufs=2) as fqpool,
                    tc.tile_pool(name="psfc", bufs=2, space="PSUM") as pfcpool,
                    tc.tile_pool(name="outp", bufs=2) as outpool,
                ):
                    for bb in range(NB):
                        rb = rel[bb]
                        # r2 = rel*alpha + MAGIC  (ACT fma -> RNE round)
                        r2 = fqpool.tile([P, 768], f32, tag="r2")
                        nc.scalar.activation(
                            out=r2[:, :], in_=rb[:, :], func=AF.Identity,
                            bias=magic[:, 0:1], scale=alpha[:, 0:1],
                        )
                        fq8 = fqpool.tile([P, 768], f8, tag="fq8")
                        nc.vector.tensor_scalar(
                            out=fq8[:, :], in0=r2[:, :],
                            scalar1=MAGIC, scalar2=None, op0=OP.subtract,
                        )
                        psfc = pfcpool.tile([10, 128], f32)
                        fqv = fq8[:, :].rearrange(
                            "p (ch n) -> p ch n", ch=6)
                        fwv = fwsb[:, :].rearrange(
                            "p (ch cls) -> p ch cls", ch=6)
                        for ch in range(6):
                            nc.tensor.matmul(
                                out=psfc[:, :],
                                lhsT=fwv[:, ch],
                                rhs=fqv[:, ch],
                                start=(ch == 0), stop=(ch == 5),
                            )
                        osb = outpool.tile([10, 128], f32)
                        nc.scalar.activation(
                            out=osb[:, :], in_=psfc[:, :], func=AF.Identity,
                            bias=fb[0:10, 0:1], scale=sprod[0:10, 0:1],
                        )
                        nc.sync.dma_start(
                            out=out_ext[:, bb * 128:(bb + 1) * 128],
                            in_=osb[:, :])

    nc.finalize()
    return nc


def _host_constants(x, conv_w, conv_b, fc_w, fc_b):
    s_x = _host_quant_scale(x)
    s_w = _host_quant_scale(conv_w)
    s_fw = _host_quant_scale(fc_w)
    kw = np.round(conv_w.astype(np.float32) / s_w).astype(np.float32)
    kfw = np.round(fc_w.astype(np.float32) / s_fw).astype(np.float32)

    # Banded DoubleRow conv weights.
    # logical row r = 28*dj + h (84); packed (i2 = r // 42, kp = r % 42).
    # m column (per c): oc*8 + t*4 + u; i = 8c + 4t + u; di = h - i in [0,3).
    w3 = np.zeros((42, 2, 3, 128), np.float32)
    for dj in range(3):
        for h in range(28):
            r = 28 * dj + h
            i2, kp = divmod(r, 42)
            for c in range(3):
                for t in range(2):
                    for u in range(4):
                        i = 8 * c + 4 * t + u
                        di = h - i
                        if 0 <= di <= 2:
                            for oc in range(16):
                                w3[kp, i2, c, oc * 8 + t * 4 + u] = \
                                    kw[oc, 0, di, dj]
    # lhsT layout [42, (c, two, m)]: DoubleRow needs the two k-halves
    # contiguous (two-stride == m) per the s3_lw_dual_fp8 ISA restriction.
    w3 = w3.transpose(0, 2, 1, 3).reshape(42, 768)

    # FC slabs permuted to the fqT layout.
    # fqT partition p = ocg*32 + jl*8 + ocl*2 + t ; chunk ch = c*2 + jh.
    # feature flat idx (torch flatten) = oc*36 + ii*6 + jj,
    #   oc = ocg*4 + ocl, ii = 2c + t, jj = jh*4 + jl (pad if jj >= 6).
    fw = np.zeros((128, 3, 2, 10), np.float32)
    for p in range(128):
        ocg, rem = divmod(p, 32)
        jl, rem2 = divmod(rem, 8)
        ocl, t = divmod(rem2, 2)
        oc = ocg * 4 + ocl
        for ch in range(6):
            c, jh = divmod(ch, 2)
            jj = jh * 4 + jl
            if jj < 6:
                k = oc * 36 + (2 * c + t) * 6 + jj
                fw[p, c, jh, :] = kfw[:, k]
    fw = fw.reshape(128, 60)

    # per-partition conv bias in int units (pad rows 0)
    cbp = np.zeros((128, 1), np.float32)
    s_xw = _f32(s_x * s_w)
    for p in range(128):
        ocg, rem = divmod(p, 32)
        jl, rem2 = divmod(rem, 8)
        ocl, t = divmod(rem2, 2)
        oc = ocg * 4 + ocl
        cbp[p, 0] = _f32(conv_b[oc] / s_xw)

    fb = np.zeros((P, 1), np.float32)
    fb[:10, 0] = fc_b.astype(np.float32)

    inv_sx = _f32(_f32(1.0) / s_x)
    scal = np.tile(
        np.array([inv_sx, s_xw, s_fw, 0.0], np.float32)[None, :], (P, 1))

    import ml_dtypes
    return {
        "w3": w3.astype(ml_dtypes.float8_e4m3),
        "fw": fw.astype(ml_dtypes.float8_e4m3),
        "cbp": cbp,
        "fb": fb,
        "scal": scal.astype(np.float32),
    }


def _get_nc():
    global _NC
    if _NC is None:
        _NC = _build_nc()
    return _NC


def kernel(x, conv_w, conv_b, fc_w, fc_b, _trace=False):
    from concourse.bass_utils import run_bass_kernel_spmd

    x = np.asarray(x, np.float32)
    consts = _host_constants(
        x, np.asarray(conv_w, np.float32), np.asarray(conv_b, np.float32),
        np.asarray(fc_w, np.float32), np.asarray(fc_b, np.float32))

    nc = _get_nc()
    in_maps = []
    for cix in range(NCORES):
        shard = x[cix * B_CORE:(cix + 1) * B_CORE]  # [1024,1,28,28]
        # h-major: [bq4, h28, b256, w28] -> [112, 7168]
        xh = shard.reshape(4, 256, 28, 28).transpose(0, 2, 1, 3)
        m = {"x": np.ascontiguousarray(xh.reshape(112, 7168))}
        m.update(consts)
        in_maps.append(m)

    res = run_bass_kernel_spmd(nc, in_maps, list(range(NCORES)), trace=_trace)
    out = np.concatenate([r["out"].T for r in res.results], axis=0)
    if _trace:
        kernel._last_results = res
    return np.ascontiguousarray(out.astype(np.float32))
